# revision 2
# baseline (speedup 1.0000x reference)
"""KStoNet (RBF-SVR heads + MLP) fused Trainium2 kernel, data-parallel over 8 cores.

v2: row-tiled stage-1 (contraction 64, two concurrent 64x128 matmuls),
exp split across Scalar (exact) + Vector (Schraudolph int16 bitcast) engines,
|x|^2 term applied as a per-column scale after stage 2 instead of extra
contraction rows.
"""
import sys

sys.path.insert(0, "/opt/trn_rl_repo")

import contextlib
import ctypes
import types

import numpy as np


def _install_axon_shims():
    """(1) NTFF profile hook this image's antenv lacks; (2) split the final SP
    Drain's sem waits (this walrus build allows only one sync wait there)."""
    if "antenv.axon_hooks" not in sys.modules:
        lib = ctypes.CDLL("/opt/axon/libaxon_pjrt.so")
        hook = None
        if hasattr(lib, "axon_start_nrt_profile"):
            lib.axon_start_nrt_profile.argtypes = [
                ctypes.POINTER(ctypes.c_int64),
                ctypes.c_size_t,
            ]
            lib.axon_start_nrt_profile.restype = ctypes.c_int64
            lib.axon_stop_nrt_profile.argtypes = [ctypes.c_char_p]
            lib.axon_stop_nrt_profile.restype = ctypes.c_int64

            @contextlib.contextmanager
            def _hook(output_dir, device_ids=None):
                import jax

                jax.devices()
                if device_ids:
                    ids = (ctypes.c_int64 * len(device_ids))(*device_ids)
                    rc = lib.axon_start_nrt_profile(ids, len(device_ids))
                else:
                    rc = lib.axon_start_nrt_profile(None, 0)
                if rc != 0:
                    raise RuntimeError(f"axon_start_nrt_profile rc={rc}")
                try:
                    yield
                finally:
                    n = lib.axon_stop_nrt_profile(str(output_dir).encode())
                    print(f"profile: {n} file(s) -> {output_dir}", file=sys.stderr)

            hook = _hook
        mod = types.ModuleType("antenv.axon_hooks")
        mod.get_axon_ntff_profile_hook = lambda: hook
        mod.set_axon_ntff_profile_hook = lambda h: None
        sys.modules["antenv.axon_hooks"] = mod
        import antenv

        antenv.axon_hooks = mod

    import bass_rust
    import concourse.tile as tile
    from concourse.vector_clock import ScopedClock

    if not getattr(tile.TileContext._drain_and_barrier, "_wait_split", False):

        def _drain_and_barrier(self, tick_clock, wait_clock):
            drain_inst = self.nc.sync.drain()
            wait_clock.add_sem_waits(
                drain_inst.ins, ScopedClock({None: tick_clock.global_clock})
            )
            si = drain_inst.ins.sync_info
            waits = list(si.on_wait) if si and si.on_wait else []
            if len(waits) > 1:
                si.on_wait = waits[:1]
                for w in waits[1:]:
                    extra = self.nc.sync.drain()
                    extra.ins.sync_info = bass_rust.SyncInfo(on_wait=[w], on_update=[])
            self.nc.all_engine_barrier()
            assert self.sems is not None
            popped = self.nc._tile_sem_poison_stack.pop()
            assert popped is self._sem_poison
            self.nc.clear_and_free_semaphores(list(self.sems.allocated().values()))
            self.nc.all_engine_barrier()

        _drain_and_barrier._wait_split = True
        tile.TileContext._drain_and_barrier = _drain_and_barrier


_install_axon_shims()

import ml_dtypes
import concourse.bass as bass
import concourse.tile as tile
from concourse import bacc, mybir
from concourse.bass_utils import run_bass_kernel_spmd

GAMMA = 0.1
B, D, H0, K = 16384, 64, 256, 50
HK = H0 * K  # 12800
NCORES = 8
BC = B // NCORES  # 2048 batch rows per core
SLAB = 512
NSLAB = BC // SLAB
NCHUNK = HK // 128  # 100 chunks of 128 (head,k) pairs
NPAIR = NCHUNK // 2  # 50 row-tiled pairs
BF16 = mybir.dt.bfloat16
F32 = mybir.dt.float32

# exp-engine split: fraction of chunk-pairs handled by the Scalar engine
# (exact exp); the rest go to the Vector engine (Schraudolph bitcast exp).
ACT_FRAC = 27.0 / 50.0
A16 = 128.0 / float(np.log(2.0))  # Schraudolph scale for bf16 bitcast
C16 = 16256.0 - 5.44  # bias (kappa=0) minus minimax correction


def _act_pair_mask():
    mask = []
    accum = 0.0
    for _ in range(NPAIR):
        accum += ACT_FRAC
        if accum >= 1.0:
            mask.append(True)
            accum -= 1.0
        else:
            mask.append(False)
    return mask


_CACHE = {}


def _build_program():
    nc = bacc.Bacc("TRN2", target_bir_lowering=False, debug=False)
    xrep_d = nc.dram_tensor("xrep", [128, BC], BF16, kind="ExternalInput")
    caug2_d = nc.dram_tensor("caug2", [128, NPAIR * 128], BF16, kind="ExternalInput")
    wmb_d = nc.dram_tensor("wmb", [128, NCHUNK * 128], BF16, kind="ExternalInput")
    xsc_d = nc.dram_tensor("xsc", [128, BC], BF16, kind="ExternalInput")
    svrb_d = nc.dram_tensor("svrb", [128, 2], F32, kind="ExternalInput")
    fcb_d = nc.dram_tensor("fcb", [128, 2], F32, kind="ExternalInput")
    fcT_d = nc.dram_tensor("fcT", [H0, H0], BF16, kind="ExternalInput")
    owT_d = nc.dram_tensor("owT", [H0, 1], BF16, kind="ExternalInput")
    out_d = nc.dram_tensor("out", [BC], F32, kind="ExternalOutput")

    Exp = mybir.ActivationFunctionType.Exp
    Tanh = mybir.ActivationFunctionType.Tanh
    act_pair = _act_pair_mask()

    with tile.TileContext(nc) as tc:
        with (
            tc.tile_pool(name="const", bufs=1) as constp,
            tc.tile_pool(name="cw", bufs=1) as cwp,
            tc.tile_pool(name="rbfw", bufs=4) as rbfwp,
            tc.tile_pool(name="hid", bufs=2) as hidp,
            tc.tile_pool(name="orow", bufs=2) as orowp,
            tc.tile_pool(name="pt1", bufs=2, space=bass.MemorySpace.PSUM) as pt1p,
            tc.tile_pool(name="pacc", bufs=1, space=bass.MemorySpace.PSUM) as paccp,
            tc.tile_pool(name="p34", bufs=1, space=bass.MemorySpace.PSUM) as p34p,
        ):
            # ---- constant loads ----
            xrep_sb = constp.tile([128, BC], BF16, tag="xrep")
            nc.sync.dma_start(xrep_sb[:], xrep_d.ap())
            # caug2 pieces (pairs) and wmb pieces (chunks); small first pieces
            CPIECES = [2, 8, 20, 20]  # pairs, sums to NPAIR
            cpiece_of = []
            for i, npc in enumerate(CPIECES):
                for j in range(npc):
                    cpiece_of.append((i, j))
            caug_sb = []
            coff = 0
            for i, npc in enumerate(CPIECES):
                ct = cwp.tile([128, npc * 128], BF16, tag=f"caug{i}", name=f"caug{i}")
                nc.sync.dma_start(
                    ct[:], caug2_d.ap()[:, coff * 128 : (coff + npc) * 128]
                )
                caug_sb.append(ct)
                coff += npc
            WPIECES = [4, 16, 40, 40]  # chunks, sums to NCHUNK
            wpiece_of = []
            for i, npc in enumerate(WPIECES):
                for j in range(npc):
                    wpiece_of.append((i, j))
            wm_sb = []
            woff = 0
            for i, npc in enumerate(WPIECES):
                wt = cwp.tile([128, npc * 128], BF16, tag=f"wm{i}", name=f"wm{i}")
                nc.sync.dma_start(
                    wt[:], wmb_d.ap()[:, woff * 128 : (woff + npc) * 128]
                )
                wm_sb.append(wt)
                woff += npc
            xsc_sb = constp.tile([128, BC], BF16, tag="xsc")
            nc.sync.dma_start(xsc_sb[:], xsc_d.ap())
            svrb_sb = constp.tile([128, 2], F32, tag="svrb")
            nc.sync.dma_start(svrb_sb[:], svrb_d.ap())
            fcb_sb = constp.tile([128, 2], F32, tag="fcb")
            nc.sync.dma_start(fcb_sb[:], fcb_d.ap())
            fcT_sb = []
            for hh in range(2):
                ft = constp.tile([128, H0], BF16, tag=f"fcT{hh}")
                nc.sync.dma_start(ft[:], fcT_d.ap()[hh * 128 : (hh + 1) * 128, :])
                fcT_sb.append(ft)
            owT_sb = []
            for hh in range(2):
                ot = constp.tile([128, 1], BF16, tag=f"owT{hh}")
                nc.sync.dma_start(ot[:], owT_d.ap()[hh * 128 : (hh + 1) * 128, :])
                owT_sb.append(ot)

            def caug_ap(t, half):
                i, j = cpiece_of[t]
                return caug_sb[i][
                    half * 64 : (half + 1) * 64, j * 128 : (j + 1) * 128
                ]

            def wm_ap(c):
                i, j = wpiece_of[c]
                return wm_sb[i][:, j * 128 : (j + 1) * 128]

            # ---- main loop ----
            for s in range(NSLAB):
                acc = [None, None]
                hidT = [None, None]
                for t in range(NPAIR):
                    # stage 1: two concurrent row-tiled matmuls (contraction 64)
                    pt = pt1p.tile([128, 2 * SLAB], F32, tag="pt1")
                    for j in range(2):
                        nc.tensor.matmul(
                            pt[:, j * SLAB : (j + 1) * SLAB],
                            caug_ap(t, j),
                            xrep_sb[j * 64 : (j + 1) * 64, s * SLAB : (s + 1) * SLAB],
                            start=True,
                            stop=True,
                        )
                    # exp on one of the two elementwise engines
                    rb = rbfwp.tile([128, 2 * SLAB], BF16, tag="rb")
                    if act_pair[t]:
                        nc.scalar.activation(rb[:], pt[:], Exp)
                    else:
                        nc.vector.tensor_scalar(
                            out=rb[:].bitcast(mybir.dt.int16),
                            in0=pt[:],
                            scalar1=A16,
                            scalar2=C16,
                            op0=mybir.AluOpType.mult,
                            op1=mybir.AluOpType.add,
                        )
                    # stage 2: accumulate head sums
                    for j in range(2):
                        c = 2 * t + j
                        half = c // (NCHUNK // 2)
                        if c % (NCHUNK // 2) == 0:
                            acc[half] = paccp.tile(
                                [128, SLAB], F32, tag="acc", name=f"acc{half}"
                            )
                        nc.tensor.matmul(
                            acc[half][:],
                            wm_ap(c),
                            rb[:, j * SLAB : (j + 1) * SLAB],
                            start=(c % (NCHUNK // 2) == 0),
                            stop=(c % (NCHUNK // 2) == (NCHUNK // 2 - 1)),
                        )
                        if c % (NCHUNK // 2) == NCHUNK // 2 - 1:
                            # hidden = tanh(acc * exp(-g*|x|^2) + svr_b)
                            hpre = hidp.tile([128, SLAB], F32, tag="hpre")
                            nc.vector.scalar_tensor_tensor(
                                hpre[:],
                                acc[half][:],
                                0.0,
                                xsc_sb[:, s * SLAB : (s + 1) * SLAB],
                                mybir.AluOpType.bypass,
                                mybir.AluOpType.mult,
                            )
                            ht = hidp.tile([128, SLAB], BF16, tag="hidT")
                            nc.scalar.activation(
                                ht[:],
                                hpre[:],
                                Tanh,
                                bias=svrb_sb[:, half : half + 1],
                            )
                            hidT[half] = ht
                # stage 3: hidden2T = tanh(fcT.T-blocks @ hidT + fcb)
                h2T = [None, None]
                for jh in range(2):
                    psB = p34p.tile([128, SLAB], F32, tag="p34")
                    for hh in range(2):
                        nc.tensor.matmul(
                            psB[:],
                            fcT_sb[hh][:, jh * 128 : (jh + 1) * 128],
                            hidT[hh][:],
                            start=(hh == 0),
                            stop=(hh == 1),
                        )
                    h2 = hidp.tile([128, SLAB], BF16, tag="h2T")
                    nc.scalar.activation(
                        h2[:], psB[:], Tanh, bias=fcb_sb[:, jh : jh + 1]
                    )
                    h2T[jh] = h2
                # stage 4: out = owT.T @ h2T + out_b  (out_b added on host)
                psC = p34p.tile([1, SLAB], F32, tag="p34", name="psC")
                for jh in range(2):
                    nc.tensor.matmul(
                        psC[:],
                        owT_sb[jh][:],
                        h2T[jh][:],
                        start=(jh == 0),
                        stop=(jh == 1),
                    )
                orow = orowp.tile([1, SLAB], F32, tag="orow")
                nc.vector.tensor_copy(orow[:], psC[:])
                nc.sync.dma_start(out_d.ap()[s * SLAB : (s + 1) * SLAB], orow[0:1, :])
    nc.compile()
    return nc


def _prep_inputs(x, centers, svr_w, svr_b, fc_w, fc_b, out_w, out_b):
    bf16 = ml_dtypes.bfloat16
    x = np.asarray(x, np.float32)
    centers = np.asarray(centers, np.float32)
    x2 = (x * x).sum(-1)  # [B]
    # xrep: x.T duplicated into both 64-row halves (for row-tiled stage 1)
    xT = x.T.astype(bf16)  # [64, B]
    xrep = np.concatenate([xT, xT], axis=0)  # [128, B]
    # caug2: per pair t, rows 0..63 = 2*gamma*cT of chunk 2t, rows 64..127 of 2t+1
    cfl = centers.reshape(HK, D)
    caugT = (2.0 * GAMMA * cfl).T.astype(bf16)  # [64, HK]
    v = caugT.reshape(D, NPAIR, 2, 128)
    caug2 = np.concatenate([v[:, :, 0, :], v[:, :, 1, :]], axis=0)  # [128, NPAIR, 128]
    caug2 = np.ascontiguousarray(caug2.reshape(128, NPAIR * 128))
    # stage-2 weights: wmb[p, c*128 + h%128] = svr_w[h,k]*exp(-gamma*c2[h,k])
    c2 = (cfl * cfl).sum(-1)  # [HK]
    wfold = (np.asarray(svr_w, np.float32).reshape(HK) * np.exp(-GAMMA * c2)).astype(
        np.float32
    )
    hk = np.arange(HK)
    heads = hk // K
    wmb = np.zeros((128, NCHUNK * 128), np.float32)
    p = hk % 128
    chunk = hk // 128
    col = chunk * 128 + (heads % 128)
    wmb[p, col] = wfold
    wmb = wmb.astype(bf16)
    # per-column scale: exp(-gamma*|x|^2), replicated to 128 partitions
    xsc_row = np.exp(-GAMMA * x2).astype(bf16)  # [B]
    xsc = np.ascontiguousarray(np.broadcast_to(xsc_row[None, :], (128, B)))
    svrb = np.stack(
        [np.asarray(svr_b, np.float32)[:128], np.asarray(svr_b, np.float32)[128:]], 1
    )
    fcb = np.stack(
        [np.asarray(fc_b, np.float32)[:128], np.asarray(fc_b, np.float32)[128:]], 1
    )
    fcT = np.ascontiguousarray(np.asarray(fc_w, np.float32).T.astype(bf16))  # [h, j]
    owT = np.ascontiguousarray(np.asarray(out_w, np.float32).T.astype(bf16))  # [h, 1]
    return xrep, caug2, wmb, xsc, svrb, fcb, fcT, owT, float(np.asarray(out_b)[0])


def kernel(x, centers, svr_w, svr_b, fc_w, fc_b, out_w, out_b, _trace=False):
    if "nc" not in _CACHE:
        _CACHE["nc"] = _build_program()
    nc = _CACHE["nc"]
    xrep, caug2, wmb, xsc, svrb, fcb, fcT, owT, ob = _prep_inputs(
        x, centers, svr_w, svr_b, fc_w, fc_b, out_w, out_b
    )
    in_maps = []
    for c in range(NCORES):
        in_maps.append(
            {
                "xrep": np.ascontiguousarray(xrep[:, c * BC : (c + 1) * BC]),
                "caug2": caug2,
                "wmb": wmb,
                "xsc": np.ascontiguousarray(xsc[:, c * BC : (c + 1) * BC]),
                "svrb": svrb,
                "fcb": fcb,
                "fcT": fcT,
                "owT": owT,
            }
        )
    res = run_bass_kernel_spmd(nc, in_maps, list(range(NCORES)), trace=_trace)
    out = np.concatenate([res.results[c]["out"] for c in range(NCORES)])
    out = (out + ob).astype(np.float32).reshape(B, 1)
    if _trace:
        kernel._last_results = res
    return out


# revision 5
# speedup vs baseline: 1.5789x; 1.5789x over previous
"""KStoNet (RBF-SVR heads + MLP) fused Trainium2 kernel, data-parallel over 8 cores.

v2: row-tiled stage-1 (contraction 64, two concurrent 64x128 matmuls),
exp split across Scalar (exact) + Vector (Schraudolph int16 bitcast) engines,
|x|^2 term applied as a per-column scale after stage 2 instead of extra
contraction rows.
"""
import sys

sys.path.insert(0, "/opt/trn_rl_repo")

import contextlib
import ctypes
import types

import numpy as np


def _install_axon_shims():
    """(1) NTFF profile hook this image's antenv lacks; (2) split the final SP
    Drain's sem waits (this walrus build allows only one sync wait there)."""
    if "antenv.axon_hooks" not in sys.modules:
        lib = ctypes.CDLL("/opt/axon/libaxon_pjrt.so")
        hook = None
        if hasattr(lib, "axon_start_nrt_profile"):
            lib.axon_start_nrt_profile.argtypes = [
                ctypes.POINTER(ctypes.c_int64),
                ctypes.c_size_t,
            ]
            lib.axon_start_nrt_profile.restype = ctypes.c_int64
            lib.axon_stop_nrt_profile.argtypes = [ctypes.c_char_p]
            lib.axon_stop_nrt_profile.restype = ctypes.c_int64

            @contextlib.contextmanager
            def _hook(output_dir, device_ids=None):
                import jax

                jax.devices()
                if device_ids:
                    ids = (ctypes.c_int64 * len(device_ids))(*device_ids)
                    rc = lib.axon_start_nrt_profile(ids, len(device_ids))
                else:
                    rc = lib.axon_start_nrt_profile(None, 0)
                if rc != 0:
                    raise RuntimeError(f"axon_start_nrt_profile rc={rc}")
                try:
                    yield
                finally:
                    n = lib.axon_stop_nrt_profile(str(output_dir).encode())
                    print(f"profile: {n} file(s) -> {output_dir}", file=sys.stderr)

            hook = _hook
        mod = types.ModuleType("antenv.axon_hooks")
        mod.get_axon_ntff_profile_hook = lambda: hook
        mod.set_axon_ntff_profile_hook = lambda h: None
        sys.modules["antenv.axon_hooks"] = mod
        import antenv

        antenv.axon_hooks = mod

    import bass_rust
    import concourse.tile as tile
    from concourse.vector_clock import ScopedClock

    if not getattr(tile.TileContext._drain_and_barrier, "_wait_split", False):

        def _drain_and_barrier(self, tick_clock, wait_clock):
            drain_inst = self.nc.sync.drain()
            wait_clock.add_sem_waits(
                drain_inst.ins, ScopedClock({None: tick_clock.global_clock})
            )
            si = drain_inst.ins.sync_info
            waits = list(si.on_wait) if si and si.on_wait else []
            if len(waits) > 1:
                si.on_wait = waits[:1]
                for w in waits[1:]:
                    extra = self.nc.sync.drain()
                    extra.ins.sync_info = bass_rust.SyncInfo(on_wait=[w], on_update=[])
            self.nc.all_engine_barrier()
            assert self.sems is not None
            popped = self.nc._tile_sem_poison_stack.pop()
            assert popped is self._sem_poison
            self.nc.clear_and_free_semaphores(list(self.sems.allocated().values()))
            self.nc.all_engine_barrier()

        _drain_and_barrier._wait_split = True
        tile.TileContext._drain_and_barrier = _drain_and_barrier


_install_axon_shims()

import ml_dtypes
import concourse.bass as bass
import concourse.tile as tile
from concourse import bacc, mybir
from concourse.bass_utils import run_bass_kernel_spmd

GAMMA = 0.1
B, D, H0, K = 16384, 64, 256, 50
HK = H0 * K  # 12800
NCORES = 8
BC = B // NCORES  # 2048 batch rows per core
SLAB = 512
NSLAB = BC // SLAB
NCHUNK = HK // 128  # 100 chunks of 128 (head,k) pairs
NPAIR = NCHUNK // 2  # 50 row-tiled pairs
BF16 = mybir.dt.bfloat16
F32 = mybir.dt.float32

# exp-engine split: fraction of chunk-pairs handled by the Scalar engine
# (exact exp); the rest go to the Vector engine (Schraudolph bitcast exp).
ACT_FRAC = 25.0 / 50.0
LAG = 3  # stage-2 lags stage-1 by this many pairs (software pipeline)
A16 = 128.0 / float(np.log(2.0))  # Schraudolph scale for bf16 bitcast
C16 = 16256.0 - 5.44  # bias (kappa=0) minus minimax correction


def _act_pair_mask():
    mask = []
    accum = 0.0
    for _ in range(NPAIR):
        accum += ACT_FRAC
        if accum >= 1.0:
            mask.append(True)
            accum -= 1.0
        else:
            mask.append(False)
    return mask


_CACHE = {}


def _build_program():
    nc = bacc.Bacc("TRN2", target_bir_lowering=False, debug=False)
    xrep_d = nc.dram_tensor("xrep", [128, BC], BF16, kind="ExternalInput")
    caug2_d = nc.dram_tensor("caug2", [128, NPAIR * 128], BF16, kind="ExternalInput")
    wmb_d = nc.dram_tensor("wmb", [128, NCHUNK * 128], BF16, kind="ExternalInput")
    xsc_d = nc.dram_tensor("xsc", [128, BC], BF16, kind="ExternalInput")
    svrb_d = nc.dram_tensor("svrb", [128, 2], F32, kind="ExternalInput")
    fcb_d = nc.dram_tensor("fcb", [128, 2], F32, kind="ExternalInput")
    fcT_d = nc.dram_tensor("fcT", [H0, H0], BF16, kind="ExternalInput")
    owT_d = nc.dram_tensor("owT", [H0, 1], BF16, kind="ExternalInput")
    out_d = nc.dram_tensor("out", [BC], F32, kind="ExternalOutput")

    Exp = mybir.ActivationFunctionType.Exp
    Tanh = mybir.ActivationFunctionType.Tanh
    act_pair = _act_pair_mask()

    with tile.TileContext(nc) as tc:
        with (
            tc.tile_pool(name="const", bufs=1) as constp,
            tc.tile_pool(name="cw", bufs=1) as cwp,
            tc.tile_pool(name="rbfw", bufs=4) as rbfwp,
            tc.tile_pool(name="hid", bufs=2) as hidp,
            tc.tile_pool(name="orow", bufs=2) as orowp,
            tc.tile_pool(name="pt1", bufs=3, space=bass.MemorySpace.PSUM) as pt1p,
            tc.tile_pool(name="pacc", bufs=1, space=bass.MemorySpace.PSUM) as paccp,
        ):
            # ---- constant loads ----
            xrep_sb = constp.tile([128, BC], BF16, tag="xrep")
            nc.sync.dma_start(xrep_sb[:], xrep_d.ap())
            # caug2 pieces (pairs) and wmb pieces (chunks); small first pieces
            CPIECES = [2, 8, 20, 20]  # pairs, sums to NPAIR
            cpiece_of = []
            for i, npc in enumerate(CPIECES):
                for j in range(npc):
                    cpiece_of.append((i, j))
            caug_sb = []
            coff = 0
            for i, npc in enumerate(CPIECES):
                ct = cwp.tile([128, npc * 128], BF16, tag=f"caug{i}", name=f"caug{i}")
                nc.sync.dma_start(
                    ct[:], caug2_d.ap()[:, coff * 128 : (coff + npc) * 128]
                )
                caug_sb.append(ct)
                coff += npc
            WPIECES = [4, 16, 40, 40]  # chunks, sums to NCHUNK
            wpiece_of = []
            for i, npc in enumerate(WPIECES):
                for j in range(npc):
                    wpiece_of.append((i, j))
            wm_sb = []
            woff = 0
            for i, npc in enumerate(WPIECES):
                wt = cwp.tile([128, npc * 128], BF16, tag=f"wm{i}", name=f"wm{i}")
                nc.sync.dma_start(
                    wt[:], wmb_d.ap()[:, woff * 128 : (woff + npc) * 128]
                )
                wm_sb.append(wt)
                woff += npc
            xsc_sb = constp.tile([128, BC], BF16, tag="xsc")
            nc.sync.dma_start(xsc_sb[:], xsc_d.ap())
            svrb_sb = constp.tile([128, 2], F32, tag="svrb")
            nc.sync.dma_start(svrb_sb[:], svrb_d.ap())
            fcb_sb = constp.tile([128, 2], F32, tag="fcb")
            nc.sync.dma_start(fcb_sb[:], fcb_d.ap())
            fcT_sb = []
            for hh in range(2):
                ft = constp.tile([128, H0], BF16, tag=f"fcT{hh}")
                nc.sync.dma_start(ft[:], fcT_d.ap()[hh * 128 : (hh + 1) * 128, :])
                fcT_sb.append(ft)
            owT_sb = []
            for hh in range(2):
                ot = constp.tile([128, 1], BF16, tag=f"owT{hh}")
                nc.sync.dma_start(ot[:], owT_d.ap()[hh * 128 : (hh + 1) * 128, :])
                owT_sb.append(ot)

            def caug_ap(t, half):
                i, j = cpiece_of[t]
                return caug_sb[i][
                    half * 64 : (half + 1) * 64, j * 128 : (j + 1) * 128
                ]

            def wm_ap(c):
                i, j = wpiece_of[c]
                return wm_sb[i][:, j * 128 : (j + 1) * 128]

            # ---- main loop (software-pipelined: stage 2 lags stage 1 by LAG pairs) ----
            for s in range(NSLAB):
                acc = [None, None]
                hidT = [None, None]
                rbs = {}
                for t in range(NPAIR + LAG):
                    if t < NPAIR:
                        # stage 1: two concurrent row-tiled matmuls (contraction 64)
                        pt = pt1p.tile([128, 2 * SLAB], F32, tag="pt1")
                        for j in range(2):
                            nc.tensor.matmul(
                                pt[:, j * SLAB : (j + 1) * SLAB],
                                caug_ap(t, j),
                                xrep_sb[
                                    j * 64 : (j + 1) * 64, s * SLAB : (s + 1) * SLAB
                                ],
                                start=True,
                                stop=True,
                            )
                        # exp on one of the two elementwise engines
                        rb = rbfwp.tile([128, 2 * SLAB], BF16, tag="rb")
                        if act_pair[t]:
                            nc.scalar.activation(rb[:], pt[:], Exp)
                        else:
                            nc.vector.tensor_scalar(
                                out=rb[:].bitcast(mybir.dt.int16),
                                in0=pt[:],
                                scalar1=A16,
                                scalar2=C16,
                                op0=mybir.AluOpType.mult,
                                op1=mybir.AluOpType.add,
                            )
                        rbs[t] = rb
                    if t >= LAG:
                        tt = t - LAG
                        # stage 2: accumulate head sums
                        for j in range(2):
                            c = 2 * tt + j
                            half = c // (NCHUNK // 2)
                            if c % (NCHUNK // 2) == 0:
                                acc[half] = paccp.tile(
                                    [128, SLAB], F32, tag="acc", name=f"acc{half}"
                                )
                            nc.tensor.matmul(
                                acc[half][:],
                                wm_ap(c),
                                rbs[tt][:, j * SLAB : (j + 1) * SLAB],
                                start=(c % (NCHUNK // 2) == 0),
                                stop=(c % (NCHUNK // 2) == (NCHUNK // 2 - 1)),
                            )
                            if c % (NCHUNK // 2) == NCHUNK // 2 - 1:
                                # hidden = tanh(acc * exp(-g*|x|^2) + svr_b)
                                hpre = hidp.tile([128, SLAB], F32, tag="hpre")
                                nc.vector.scalar_tensor_tensor(
                                    hpre[:],
                                    acc[half][:],
                                    0.0,
                                    xsc_sb[:, s * SLAB : (s + 1) * SLAB],
                                    mybir.AluOpType.bypass,
                                    mybir.AluOpType.mult,
                                )
                                ht = hidp.tile([128, SLAB], BF16, tag="hidT")
                                nc.scalar.activation(
                                    ht[:],
                                    hpre[:],
                                    Tanh,
                                    bias=svrb_sb[:, half : half + 1],
                                )
                                hidT[half] = ht
                # stage 3: hidden2T = tanh(fcT.T-blocks @ hidT + fcb)
                # psum for stages 3/4 comes from the pt1 pool (reuse rotation)
                psAB = pt1p.tile([128, 2 * SLAB], F32, tag="pt1", name="psAB")
                h2T = [None, None]
                for jh in range(2):
                    psB = psAB[:, jh * SLAB : (jh + 1) * SLAB]
                    for hh in range(2):
                        nc.tensor.matmul(
                            psB,
                            fcT_sb[hh][:, jh * 128 : (jh + 1) * 128],
                            hidT[hh][:],
                            start=(hh == 0),
                            stop=(hh == 1),
                        )
                    h2 = hidp.tile([128, SLAB], BF16, tag="h2T")
                    nc.scalar.activation(
                        h2[:], psB, Tanh, bias=fcb_sb[:, jh : jh + 1]
                    )
                    h2T[jh] = h2
                # stage 4: out = owT.T @ h2T + out_b  (out_b added on host)
                psCt = pt1p.tile([128, 2 * SLAB], F32, tag="pt1", name="psCt")
                psC = psCt[0:1, 0:SLAB]
                for jh in range(2):
                    nc.tensor.matmul(
                        psC,
                        owT_sb[jh][:],
                        h2T[jh][:],
                        start=(jh == 0),
                        stop=(jh == 1),
                    )
                orow = orowp.tile([1, SLAB], F32, tag="orow")
                nc.vector.tensor_copy(orow[:], psC)
                nc.sync.dma_start(out_d.ap()[s * SLAB : (s + 1) * SLAB], orow[0:1, :])
    nc.compile()
    return nc


def _prep_inputs(x, centers, svr_w, svr_b, fc_w, fc_b, out_w, out_b):
    bf16 = ml_dtypes.bfloat16
    x = np.asarray(x, np.float32)
    centers = np.asarray(centers, np.float32)
    x2 = (x * x).sum(-1)  # [B]
    # xrep: x.T duplicated into both 64-row halves (for row-tiled stage 1)
    xT = x.T.astype(bf16)  # [64, B]
    xrep = np.concatenate([xT, xT], axis=0)  # [128, B]
    # caug2: per pair t, rows 0..63 = 2*gamma*cT of chunk 2t, rows 64..127 of 2t+1
    cfl = centers.reshape(HK, D)
    caugT = (2.0 * GAMMA * cfl).T.astype(bf16)  # [64, HK]
    v = caugT.reshape(D, NPAIR, 2, 128)
    caug2 = np.concatenate([v[:, :, 0, :], v[:, :, 1, :]], axis=0)  # [128, NPAIR, 128]
    caug2 = np.ascontiguousarray(caug2.reshape(128, NPAIR * 128))
    # stage-2 weights: wmb[p, c*128 + h%128] = svr_w[h,k]*exp(-gamma*c2[h,k])
    c2 = (cfl * cfl).sum(-1)  # [HK]
    wfold = (np.asarray(svr_w, np.float32).reshape(HK) * np.exp(-GAMMA * c2)).astype(
        np.float32
    )
    hk = np.arange(HK)
    heads = hk // K
    wmb = np.zeros((128, NCHUNK * 128), np.float32)
    p = hk % 128
    chunk = hk // 128
    col = chunk * 128 + (heads % 128)
    wmb[p, col] = wfold
    wmb = wmb.astype(bf16)
    # per-column scale: exp(-gamma*|x|^2), replicated to 128 partitions
    xsc_row = np.exp(-GAMMA * x2).astype(bf16)  # [B]
    xsc = np.ascontiguousarray(np.broadcast_to(xsc_row[None, :], (128, B)))
    svrb = np.stack(
        [np.asarray(svr_b, np.float32)[:128], np.asarray(svr_b, np.float32)[128:]], 1
    )
    fcb = np.stack(
        [np.asarray(fc_b, np.float32)[:128], np.asarray(fc_b, np.float32)[128:]], 1
    )
    fcT = np.ascontiguousarray(np.asarray(fc_w, np.float32).T.astype(bf16))  # [h, j]
    owT = np.ascontiguousarray(np.asarray(out_w, np.float32).T.astype(bf16))  # [h, 1]
    return xrep, caug2, wmb, xsc, svrb, fcb, fcT, owT, float(np.asarray(out_b)[0])


def kernel(x, centers, svr_w, svr_b, fc_w, fc_b, out_w, out_b, _trace=False):
    if "nc" not in _CACHE:
        _CACHE["nc"] = _build_program()
    nc = _CACHE["nc"]
    xrep, caug2, wmb, xsc, svrb, fcb, fcT, owT, ob = _prep_inputs(
        x, centers, svr_w, svr_b, fc_w, fc_b, out_w, out_b
    )
    in_maps = []
    for c in range(NCORES):
        in_maps.append(
            {
                "xrep": np.ascontiguousarray(xrep[:, c * BC : (c + 1) * BC]),
                "caug2": caug2,
                "wmb": wmb,
                "xsc": np.ascontiguousarray(xsc[:, c * BC : (c + 1) * BC]),
                "svrb": svrb,
                "fcb": fcb,
                "fcT": fcT,
                "owT": owT,
            }
        )
    res = run_bass_kernel_spmd(nc, in_maps, list(range(NCORES)), trace=_trace)
    out = np.concatenate([res.results[c]["out"] for c in range(NCORES)])
    out = (out + ob).astype(np.float32).reshape(B, 1)
    if _trace:
        kernel._last_results = res
    return out


# revision 15
# speedup vs baseline: 1.9025x; 1.2049x over previous
"""KStoNet (RBF-SVR heads + MLP) fused Trainium2 kernel, data-parallel over 8 cores.

v4: row-tiled stage-1 (contraction 64, two concurrent 64x128 matmuls);
exp split across Scalar (exact exp -> fp8) + Vector (Schraudolph int16
bitcast -> bf16) engines; stage-2 via fp8 DoubleRow matmuls for the
Scalar-engine pairs and bf16 matmuls for the Vector-engine pairs;
|x|^2 term applied as a per-column scale after stage 2. Software
pipelined in 2-pair groups to keep all three engines busy.
"""
import sys

sys.path.insert(0, "/opt/trn_rl_repo")

import contextlib
import ctypes
import types

import numpy as np


def _install_axon_shims():
    """(1) NTFF profile hook this image's antenv lacks; (2) split the final SP
    Drain's sem waits (this walrus build allows only one sync wait there)."""
    if "antenv.axon_hooks" not in sys.modules:
        lib = ctypes.CDLL("/opt/axon/libaxon_pjrt.so")
        hook = None
        if hasattr(lib, "axon_start_nrt_profile"):
            lib.axon_start_nrt_profile.argtypes = [
                ctypes.POINTER(ctypes.c_int64),
                ctypes.c_size_t,
            ]
            lib.axon_start_nrt_profile.restype = ctypes.c_int64
            lib.axon_stop_nrt_profile.argtypes = [ctypes.c_char_p]
            lib.axon_stop_nrt_profile.restype = ctypes.c_int64

            @contextlib.contextmanager
            def _hook(output_dir, device_ids=None):
                import jax

                jax.devices()
                if device_ids:
                    ids = (ctypes.c_int64 * len(device_ids))(*device_ids)
                    rc = lib.axon_start_nrt_profile(ids, len(device_ids))
                else:
                    rc = lib.axon_start_nrt_profile(None, 0)
                if rc != 0:
                    raise RuntimeError(f"axon_start_nrt_profile rc={rc}")
                try:
                    yield
                finally:
                    n = lib.axon_stop_nrt_profile(str(output_dir).encode())
                    print(f"profile: {n} file(s) -> {output_dir}", file=sys.stderr)

            hook = _hook
        mod = types.ModuleType("antenv.axon_hooks")
        mod.get_axon_ntff_profile_hook = lambda: hook
        mod.set_axon_ntff_profile_hook = lambda h: None
        sys.modules["antenv.axon_hooks"] = mod
        import antenv

        antenv.axon_hooks = mod

    import bass_rust
    import concourse.tile as tile
    from concourse.vector_clock import ScopedClock

    if not getattr(tile.TileContext._drain_and_barrier, "_wait_split", False):

        def _drain_and_barrier(self, tick_clock, wait_clock):
            drain_inst = self.nc.sync.drain()
            wait_clock.add_sem_waits(
                drain_inst.ins, ScopedClock({None: tick_clock.global_clock})
            )
            si = drain_inst.ins.sync_info
            waits = list(si.on_wait) if si and si.on_wait else []
            if len(waits) > 1:
                si.on_wait = waits[:1]
                for w in waits[1:]:
                    extra = self.nc.sync.drain()
                    extra.ins.sync_info = bass_rust.SyncInfo(on_wait=[w], on_update=[])
            self.nc.all_engine_barrier()
            assert self.sems is not None
            popped = self.nc._tile_sem_poison_stack.pop()
            assert popped is self._sem_poison
            self.nc.clear_and_free_semaphores(list(self.sems.allocated().values()))
            self.nc.all_engine_barrier()

        _drain_and_barrier._wait_split = True
        tile.TileContext._drain_and_barrier = _drain_and_barrier


_install_axon_shims()

import ml_dtypes
import concourse.bass as bass
import concourse.tile as tile
from concourse import bacc, mybir
from concourse.bass_utils import run_bass_kernel_spmd

GAMMA = 0.1
B, D, H0, K = 16384, 64, 256, 50
HK = H0 * K  # 12800
NCORES = 8
BC = B // NCORES  # 2048 batch rows per core
SLAB = 512
NSLAB = BC // SLAB
NCHUNK = HK // 128  # 100 chunks of 128 (head,k) pairs
NPAIR = NCHUNK // 2  # 50 row-tiled pairs
ACT_EXTRA = ()  # odd pairs promoted to the Scalar engine
NHALF = NCHUNK // 2  # chunks per head-half accumulation group
LAGP = 4  # stage-2 lags stage-1 by this many pairs (software pipeline)
BF16 = mybir.dt.bfloat16
FP8 = mybir.dt.float8e4
F32 = mybir.dt.float32

LN2 = float(np.log(2.0))
A16 = 128.0 / LN2  # Schraudolph scale for bf16 bitcast
C16 = 16256.0 - 5.44  # bias (kappa=0) minus minimax correction
A8 = 8.0 / LN2  # Schraudolph scale for fp8 bitcast
# C8 set below K8
K8 = -12.05  # log2 scale folded into the fp8 rb encoding
G = K8 + 14.4654  # global log2 scale of the stage-2 accumulator (keeps wm8 max ~200)
C8 = 56.0 + 8.0 * K8 - 0.344  # fp8 Schraudolph bias (relies on HW uint8 saturate-to-0)
W8SCALE = 2.0 ** (G - K8)
WBSCALE = 2.0**G

_CACHE = {}


def _build_program():
    nc = bacc.Bacc("TRN2", target_bir_lowering=False, debug=False)
    xrep_d = nc.dram_tensor("xrep", [128, BC], BF16, kind="ExternalInput")
    caug2_d = nc.dram_tensor("caug2", [128, NPAIR * 128], BF16, kind="ExternalInput")
    wm8_d = nc.dram_tensor("wm8", [128, NCHUNK * 128], FP8, kind="ExternalInput")
    xsc_d = nc.dram_tensor("xsc", [128, BC], BF16, kind="ExternalInput")
    k8b_d = nc.dram_tensor("k8b", [128, 1], F32, kind="ExternalInput")
    svrb_d = nc.dram_tensor("svrb", [128, 2], F32, kind="ExternalInput")
    fcb_d = nc.dram_tensor("fcb", [128, 2], F32, kind="ExternalInput")
    fcT_d = nc.dram_tensor("fcT", [H0, H0], BF16, kind="ExternalInput")
    owT_d = nc.dram_tensor("owT", [H0, 1], BF16, kind="ExternalInput")
    out_d = nc.dram_tensor("out", [BC], F32, kind="ExternalOutput")

    Exp = mybir.ActivationFunctionType.Exp
    Tanh = mybir.ActivationFunctionType.Tanh
    DR = mybir.MatmulPerfMode.DoubleRow

    with tile.TileContext(nc) as tc:
        with (
            tc.tile_pool(name="const", bufs=1) as constp,
            tc.tile_pool(name="cw", bufs=1) as cwp,
            tc.tile_pool(name="rb8", bufs=3) as rb8p,
            tc.tile_pool(name="hid", bufs=2) as hidp,
            tc.tile_pool(name="orow", bufs=2) as orowp,
            tc.tile_pool(name="pt1", bufs=3, space=bass.MemorySpace.PSUM) as pt1p,
            tc.tile_pool(name="pacc", bufs=2, space=bass.MemorySpace.PSUM) as paccp,
        ):
            # ---- constant loads (ordered so slab-0 compute starts ASAP) ----
            k8b_sb = constp.tile([128, 1], F32, tag="k8b")
            nc.sync.dma_start(k8b_sb[:], k8b_d.ap())
            svrb_sb = constp.tile([128, 2], F32, tag="svrb")
            nc.sync.dma_start(svrb_sb[:], svrb_d.ap())
            CPIECES = [2, 8, 20, 20]  # pairs, sums to NPAIR
            WPIECES = [2, 8, 20, 20]  # pairs of wm8, sums to NPAIR
            cpiece_of = []
            for i, npc in enumerate(CPIECES):
                for j in range(npc):
                    cpiece_of.append((i, j))
            w8piece_of = []
            for i, npc in enumerate(WPIECES):
                for j in range(npc):
                    w8piece_of.append((i, j))
            caug_sb = [
                cwp.tile([128, npc * 128], BF16, tag=f"caug{i}", name=f"caug{i}")
                for i, npc in enumerate(CPIECES)
            ]
            wm8_sb = [
                cwp.tile([128, npc * 2, 128], FP8, tag=f"wm8{i}", name=f"wm8{i}")
                for i, npc in enumerate(WPIECES)
            ]
            xrep_sb = [
                constp.tile([128, SLAB], BF16, tag=f"xrep{s}", name=f"xrep{s}")
                for s in range(NSLAB)
            ]
            xsc_sb = constp.tile([128, BC], BF16, tag="xsc")
            fcb_sb = constp.tile([128, 2], F32, tag="fcb")
            fcT_sb = [constp.tile([128, H0], BF16, tag=f"fcT{hh}", name=f"fcT{hh}") for hh in range(2)]
            owT_sb = [constp.tile([128, 1], BF16, tag=f"owT{hh}", name=f"owT{hh}") for hh in range(2)]
            coffs = np.cumsum([0] + CPIECES).tolist()
            woffs = np.cumsum([0] + WPIECES).tolist()

            def dma_caug(i):
                nc.sync.dma_start(
                    caug_sb[i][:],
                    caug2_d.ap()[:, coffs[i] * 128 : coffs[i + 1] * 128],
                )

            def dma_wm8(i):
                nc.sync.dma_start(
                    wm8_sb[i][:],
                    wm8_d.ap()[:, woffs[i] * 256 : woffs[i + 1] * 256],
                )

            dma_caug(0)
            nc.sync.dma_start(xrep_sb[0][:], xrep_d.ap()[:, 0:SLAB])
            dma_wm8(0)
            dma_caug(1)
            dma_wm8(1)
            nc.sync.dma_start(xsc_sb[:], xsc_d.ap())
            dma_caug(2)
            dma_wm8(2)
            nc.sync.dma_start(fcb_sb[:], fcb_d.ap())
            for hh in range(2):
                nc.sync.dma_start(
                    fcT_sb[hh][:], fcT_d.ap()[hh * 128 : (hh + 1) * 128, :]
                )
                nc.sync.dma_start(
                    owT_sb[hh][:], owT_d.ap()[hh * 128 : (hh + 1) * 128, :]
                )
            dma_caug(3)
            dma_wm8(3)
            for s in range(1, NSLAB):
                nc.sync.dma_start(
                    xrep_sb[s][:], xrep_d.ap()[:, s * SLAB : (s + 1) * SLAB]
                )

            def caug_ap(t, half):
                i, j = cpiece_of[t]
                return caug_sb[i][
                    half * 64 : (half + 1) * 64, j * 128 : (j + 1) * 128
                ]

            def wm8_ap(t):  # pair index (0..49) -> [128, 2, 128]
                i, j = w8piece_of[t]
                return wm8_sb[i][:, 2 * j : 2 * j + 2, :]

            state = {}

            def emit_s1(s, t):
                pt = pt1p.tile([128, 2 * SLAB], F32, tag="pt1")
                for j in range(2):
                    nc.tensor.matmul(
                        pt[:, j * SLAB : (j + 1) * SLAB],
                        caug_ap(t, j),
                        xrep_sb[s][j * 64 : (j + 1) * 64, :],
                        start=True,
                        stop=True,
                    )
                state[t] = pt

            def is_act(t):
                return t % 2 == 0 or t in ACT_EXTRA

            def emit_exp(t):
                pt = state[t]
                if is_act(t):
                    rb = rb8p.tile([128, 2, SLAB], FP8, tag="rb8")
                    nc.scalar.activation(rb[:, :, :], pt[:, :], Exp, bias=k8b_sb[:, 0:1])
                else:
                    rb = rb8p.tile([128, 2, SLAB], FP8, tag="rb8", name="rb8d")
                    nc.vector.tensor_scalar(
                        out=rb[:, :, :].bitcast(mybir.dt.uint8),
                        in0=pt[:],
                        scalar1=A8,
                        scalar2=C8,
                        op0=mybir.AluOpType.mult,
                        op1=mybir.AluOpType.add,
                    )
                state[t] = rb

            def emit_s2(s, t, acc, hidT):
                rb = state.pop(t)
                half = (2 * t) // NHALF
                if (2 * t) % NHALF == 0:
                    acc[half] = paccp.tile(
                        [128, SLAB], F32, tag="acc", name=f"acc{half}"
                    )
                nc.tensor.matmul(
                    acc[half][:],
                    wm8_ap(t),
                    rb[:, :, :],
                    start=((2 * t) % NHALF == 0),
                    stop=((2 * t + 1) % NHALF == NHALF - 1),
                    perf_mode=DR,
                    skip_group_check=True,
                )
                if (2 * t + 1) % NHALF == NHALF - 1:
                    # hidden = tanh(acc * exp(-g*|x|^2)*2^-G + svr_b)
                    hpre = hidp.tile([128, SLAB], F32, tag="hpre", name="hpre")
                    nc.vector.scalar_tensor_tensor(
                        hpre[:],
                        acc[half][:],
                        0.0,
                        xsc_sb[:, s * SLAB : (s + 1) * SLAB],
                        mybir.AluOpType.bypass,
                        mybir.AluOpType.mult,
                    )
                    ht = hidp.tile([128, SLAB], BF16, tag="hidT", name="ht")
                    nc.scalar.activation(
                        ht[:], hpre[:], Tanh, bias=svrb_sb[:, half : half + 1]
                    )
                    hidT[half] = ht

            def emit_stage34(s, hidT):
                psAB = pt1p.tile([128, 2 * SLAB], F32, tag="pt1", name="psAB")
                h2T = [None, None]
                for jh in range(2):
                    psB = psAB[:, jh * SLAB : (jh + 1) * SLAB]
                    for hh in range(2):
                        nc.tensor.matmul(
                            psB,
                            fcT_sb[hh][:, jh * 128 : (jh + 1) * 128],
                            hidT[hh][:],
                            start=(hh == 0),
                            stop=(hh == 1),
                        )
                    h2 = hidp.tile([128, SLAB], BF16, tag="h2T", name="h2")
                    nc.scalar.activation(
                        h2[:], psB, Tanh, bias=fcb_sb[:, jh : jh + 1]
                    )
                    h2T[jh] = h2
                psCt = pt1p.tile([128, 2 * SLAB], F32, tag="pt1", name="psCt")
                psC = psCt[0:1, 0:SLAB]
                for jh in range(2):
                    nc.tensor.matmul(
                        psC,
                        owT_sb[jh][:],
                        h2T[jh][:],
                        start=(jh == 0),
                        stop=(jh == 1),
                    )
                orow = orowp.tile([1, SLAB], F32, tag="orow", name="orow")
                nc.vector.tensor_copy(orow[:], psC)
                nc.sync.dma_start(out_d.ap()[s * SLAB : (s + 1) * SLAB], orow[0:1, :])

            # ---- main loop (software-pipelined in 2-pair groups) ----
            prev = None  # (slab, hidT) awaiting stage3/4
            for s in range(NSLAB):
                acc = [None, None]
                hidT = [None, None]
                for u in range(0, NPAIR + LAGP, 2):
                    # stage-2 first: its inputs are LAGP pairs old, so the PE
                    # always has ready work at the head of each group
                    for t in (u - LAGP, u - LAGP + 1):
                        if 0 <= t < NPAIR:
                            emit_s2(s, t, acc, hidT)
                    for t in (u, u + 1):
                        if t < NPAIR:
                            emit_s1(s, t)
                    for t in (u, u + 1):
                        if t < NPAIR:
                            emit_exp(t)
                    if u == 4 and prev is not None:
                        emit_stage34(*prev)
                        prev = None
                prev = (s, hidT)
            emit_stage34(*prev)
    nc.compile()
    return nc


def _prep_inputs(x, centers, svr_w, svr_b, fc_w, fc_b, out_w, out_b):
    bf16 = ml_dtypes.bfloat16
    fp8 = ml_dtypes.float8_e4m3fn
    x = np.asarray(x, np.float32)
    centers = np.asarray(centers, np.float32)
    x2 = (x * x).sum(-1)  # [B]
    # xrep: x.T duplicated into both 64-row halves (for row-tiled stage 1)
    xT = x.T.astype(bf16)  # [64, B]
    xrep = np.concatenate([xT, xT], axis=0)  # [128, B]
    # caug2: per pair t, rows 0..63 = 2*gamma*cT of chunk 2t, rows 64..127 of 2t+1
    cfl = centers.reshape(HK, D)
    caugT = (2.0 * GAMMA * cfl).T.astype(bf16)  # [64, HK]
    v = caugT.reshape(D, NPAIR, 2, 128)
    caug2 = np.concatenate([v[:, :, 0, :], v[:, :, 1, :]], axis=0)  # [128, NPAIR, 128]
    caug2 = np.ascontiguousarray(caug2.reshape(128, NPAIR * 128))
    # stage-2 weights: wm[p, c*128 + h%128] = svr_w[h,k]*exp(-gamma*c2[h,k])
    c2 = (cfl * cfl).sum(-1)  # [HK]
    wfold = (np.asarray(svr_w, np.float32).reshape(HK) * np.exp(-GAMMA * c2)).astype(
        np.float32
    )
    hk = np.arange(HK)
    heads = hk // K
    wm = np.zeros((128, NCHUNK * 128), np.float32)
    p = hk % 128
    chunk = hk // 128
    col = chunk * 128 + (heads % 128)
    wm[p, col] = wfold
    wm8 = (wm * W8SCALE).astype(fp8)
    assert np.isfinite(wm8.astype(np.float32)).all()
    # per-column scale: exp(-gamma*|x|^2) * 2^-G, replicated to 128 partitions
    xsc_row = (np.exp(-GAMMA * x2) * 2.0**-G).astype(bf16)  # [B]
    xsc = np.ascontiguousarray(np.broadcast_to(xsc_row[None, :], (128, B)))
    svrb = np.stack(
        [np.asarray(svr_b, np.float32)[:128], np.asarray(svr_b, np.float32)[128:]], 1
    )
    fcb = np.stack(
        [np.asarray(fc_b, np.float32)[:128], np.asarray(fc_b, np.float32)[128:]], 1
    )
    fcT = np.ascontiguousarray(np.asarray(fc_w, np.float32).T.astype(bf16))  # [h, j]
    owT = np.ascontiguousarray(np.asarray(out_w, np.float32).T.astype(bf16))  # [h, 1]
    k8b = np.full((128, 1), K8 * LN2, np.float32)
    return xrep, caug2, wm8, xsc, k8b, svrb, fcb, fcT, owT, float(np.asarray(out_b)[0])


def kernel(x, centers, svr_w, svr_b, fc_w, fc_b, out_w, out_b, _trace=False):
    if "nc" not in _CACHE:
        _CACHE["nc"] = _build_program()
    nc = _CACHE["nc"]
    xrep, caug2, wm8, xsc, k8b, svrb, fcb, fcT, owT, ob = _prep_inputs(
        x, centers, svr_w, svr_b, fc_w, fc_b, out_w, out_b
    )
    in_maps = []
    for c in range(NCORES):
        in_maps.append(
            {
                "xrep": np.ascontiguousarray(xrep[:, c * BC : (c + 1) * BC]),
                "caug2": caug2,
                "wm8": wm8,
                "xsc": np.ascontiguousarray(xsc[:, c * BC : (c + 1) * BC]),
                "k8b": k8b,
                "svrb": svrb,
                "fcb": fcb,
                "fcT": fcT,
                "owT": owT,
            }
        )
    res = run_bass_kernel_spmd(nc, in_maps, list(range(NCORES)), trace=_trace)
    out = np.concatenate([res.results[c]["out"] for c in range(NCORES)])
    out = (out + ob).astype(np.float32).reshape(B, 1)
    if _trace:
        kernel._last_results = res
    return out
